# revision 1
# baseline (speedup 1.0000x reference)
"""Trainium2 Bass kernel for nn_CustomRelation (sparse_attention).

Per batch b:
    qkw = hidden @ W + bias            # [S, 128] = [q(64) | k(64)]
    RoPE(qkw) (interleaved pairs)
    logits[r] = q[i0[r]].k[i2[r]] + q[i1[r]].k[i3[r]]
    out = (logits + (1 - mask) * -1e12) / 8

Distribution: data-parallel over batch, 2 batches per NeuronCore x 8 cores.

Per-core dataflow (feature-major, SBUF-resident tables):
  - X tiles cast f32->bf16, transposed by DMA-xbar (2-byte transpose) into
    XT [128h, 512t] bf16; qkw^T [128f, 512t] = sum_k W_k^T @ XT_k on PE.
  - rot(qkw) via signed pair-swap permutation matmul.
  - sin/cos from position_ids on device: pg = pos*invf/(2pi) (K=1 matmul),
    f = pg - round(pg) (magic-number round), sin = Sin(2pi f),
    cos = Sin(-2pi|f| + pi/2) (ACT Sin domain is [-pi, pi]).
  - rope = qkw*cos + rot*sin accumulated into a per-batch feature-major
    SBUF table [128, 2048]; a half-swapped copy [k|q] is made via a
    permutation matmul (aligns k features with q partitions for the dots).
  - ap_gather (GPSIMD ucode) gathers tokens along the free dim entirely
    in SBUF; dot products via elementwise mul + ones-vector matmul
    (partition reduction on PE); mask+scale on [1, 2048]; store.
"""

import numpy as np

B, S, H, D = 16, 2048, 1024, 64
NCORES = 8
BC = B // NCORES            # batches per core
T = BC * S                  # tokens per core
DH = 2 * D                  # projected features (q|k)
MT = 512                    # macro-tile tokens
NM = T // MT                # macro-tiles per core
NG = MT // 128              # 128-token groups per macro-tile
KH = H // 128               # contraction chunks
MAGIC = 1.5 * 2.0 ** 23
TWO_PI = float(2.0 * np.pi)
HALF_PI = float(0.5 * np.pi)

_CACHE = {}


def _build_nc():
    import concourse.bass as bass
    import concourse.tile as tile
    from concourse import bacc, mybir, library_config

    f32 = mybir.dt.float32
    bf16 = mybir.dt.bfloat16
    i32 = mybir.dt.int32
    i16 = mybir.dt.int16
    Alu = mybir.AluOpType
    Act = mybir.ActivationFunctionType

    nc = bacc.Bacc("TRN2", target_bir_lowering=False, debug=False,
                   num_devices=NCORES)

    x = nc.dram_tensor("x", [T, H], f32, kind="ExternalInput")
    w = nc.dram_tensor("w", [H, DH], f32, kind="ExternalInput")
    bvec = nc.dram_tensor("bvec", [DH], f32, kind="ExternalInput")
    pos = nc.dram_tensor("pos", [BC, S], i32, kind="ExternalInput")
    idx16 = nc.dram_tensor("idx16", [BC, 2, 128, S // 16], i16,
                           kind="ExternalInput")
    msk = nc.dram_tensor("msk", [BC, S], f32, kind="ExternalInput")
    out = nc.dram_tensor("out", [BC, S], f32, kind="ExternalOutput")

    # --- constants baked into the NEFF ---
    invf = np.power(10000.0, -np.arange(D // 2, dtype=np.float64) / (D / 2.0))
    invf_rep = np.repeat(invf, 2)
    g_rep = np.concatenate([invf_rep, invf_rep]) / (2 * np.pi)
    g_rep = g_rep.reshape(1, DH).astype(np.float32)     # [1,128] lhsT (K=1)
    pswapT = np.zeros((DH, DH), dtype=np.float32)       # signed pair swap
    for j in range(D):
        pswapT[2 * j + 1, 2 * j] = -1.0                 # rot[2j]   = -x[2j+1]
        pswapT[2 * j, 2 * j + 1] = 1.0                  # rot[2j+1] =  x[2j]
    sqqT = np.zeros((DH, DH), dtype=np.float32)         # [q|k] -> [q|q]
    skkT = np.zeros((DH, DH), dtype=np.float32)         # [q|k] -> [k|k]
    for dd in range(DH):
        sqqT[dd % D, dd] = 1.0
        skkT[D + dd % D, dd] = 1.0
    grep_t = nc.inline_tensor(g_rep, "grep")
    pswap_t = nc.inline_tensor(pswapT, "pswapT")
    sqq_t = nc.inline_tensor(sqqT, "sqqT")
    skk_t = nc.inline_tensor(skkT, "skkT")
    ones_t = nc.inline_tensor(np.ones((DH, 1), dtype=np.float32), "ones")

    with tile.TileContext(nc) as tc, \
         tc.tile_pool(name="consts", bufs=1) as consts, \
         tc.tile_pool(name="xp", bufs=3) as xp, \
         tc.tile_pool(name="xt", bufs=2) as xtp, \
         tc.tile_pool(name="sb", bufs=2) as sbp, \
         tc.tile_pool(name="sc", bufs=2) as scp, \
         tc.tile_pool(name="fm", bufs=1) as fmp, \
         tc.tile_pool(name="gth", bufs=1) as gth, \
         tc.tile_pool(name="ps_qkw", bufs=2, space="PSUM") as ps_qkw, \
         tc.tile_pool(name="ps_rot", bufs=1, space="PSUM") as ps_rot, \
         tc.tile_pool(name="ps_pg", bufs=1, space="PSUM") as ps_pg, \
         tc.tile_pool(name="ps_fm2", bufs=1, space="PSUM") as ps_fm2, \
         tc.tile_pool(name="ps_lg", bufs=1, space="PSUM") as ps_lg:

        nc.gpsimd.load_library(library_config.ap_gather)

        # ---- constants / small inputs ----
        grep = consts.tile([1, DH], f32, tag="grep")
        nc.sync.dma_start(out=grep[:], in_=grep_t.ap())
        pswap = consts.tile([DH, DH], f32, tag="pswap")
        nc.sync.dma_start(out=pswap[:], in_=pswap_t.ap())
        sqq = consts.tile([DH, DH], f32, tag="sqq")
        nc.sync.dma_start(out=sqq[:], in_=sqq_t.ap())
        skk = consts.tile([DH, DH], f32, tag="skk")
        nc.sync.dma_start(out=skk[:], in_=skk_t.ap())
        ones = consts.tile([DH, 1], f32, tag="ones")
        nc.sync.dma_start(out=ones[:], in_=ones_t.ap())
        bcol = consts.tile([DH, 1], f32, tag="bcol")
        nc.sync.dma_start(out=bcol[:], in_=bvec.ap().rearrange("(p o) -> p o", o=1))
        wsb = []
        for k in range(KH):
            wf = consts.tile([128, DH], f32, tag=f"wf{k}")
            nc.sync.dma_start(out=wf[:], in_=w[128 * k:128 * (k + 1), :])
            wb = consts.tile([128, DH], bf16, tag=f"wb{k}")
            nc.vector.tensor_copy(out=wb[:], in_=wf[:])
            wsb.append(wb)
        pos_fb = []
        for b in range(BC):
            pib = consts.tile([1, S], i32, tag=f"pos_i{b}")
            nc.sync.dma_start(out=pib[:], in_=pos[b:b + 1, :])
            pfb = consts.tile([1, S], f32, tag=f"pos_f{b}")
            nc.vector.tensor_copy(out=pfb[:], in_=pib[:])
            pos_fb.append(pfb)
        zerob = consts.tile([DH, 1], f32, tag="zerob")
        nc.vector.memset(zerob[:], 0.0)
        pihalf = consts.tile([DH, 1], f32, tag="pihalf")
        nc.vector.memset(pihalf[:], HALF_PI)

        # per-batch feature-major tables (SBUF-resident)
        fqq = [fmp.tile([DH, S], f32, name=f"fqq{b}", tag=f"fqq{b}")
               for b in range(BC)]
        fkk = [fmp.tile([DH, S], f32, name=f"fkk{b}", tag=f"fkk{b}")
               for b in range(BC)]

        # ---- main pipeline over macro-tiles ----
        for m in range(NM):
            bi = m // (NM // BC)
            t0 = m * MT
            tl0 = t0 - bi * S

            # load + cast + DMA-transpose into XT bf16 [128h, k, 512t]
            xtall = xtp.tile([128, KH, MT], bf16, name=f"xta_{m}", tag="xta")
            for g in range(NG):
                xg = xp.tile([128, H], f32, tag="x")
                nc.sync.dma_start(
                    out=xg[:], in_=x[t0 + 128 * g:t0 + 128 * (g + 1), :])
                xb = xp.tile([128, H], bf16, tag="xb")
                if g % 2 == 0:
                    nc.vector.tensor_copy(out=xb[:], in_=xg[:])
                else:
                    nc.scalar.copy(out=xb[:], in_=xg[:])
                # one xbar transpose for all KH chunks: row 128k+p of x^T
                # lands at out[p, k, :]
                nc.sync.dma_start_transpose(
                    out=xtall[:, :, 128 * g:128 * (g + 1)], in_=xb[:])
            xts = [xtall[:, k, :] for k in range(KH)]

            # qkw^T [128f, 512t] = sum_k W_k^T @ XT_k  (+bias)
            pq = ps_qkw.tile([DH, MT], f32, tag="qkw")
            for k in range(KH):
                nc.tensor.matmul(out=pq[:], lhsT=wsb[k][:], rhs=xts[k],
                                 start=(k == 0), stop=(k == KH - 1))
            qkw = sbp.tile([DH, MT], f32, tag="qkw_sb")
            nc.vector.tensor_scalar(out=qkw[:], in0=pq[:], scalar1=bcol[:],
                                    scalar2=None, op0=Alu.add)

            # rot(qkw)
            pr = ps_rot.tile([DH, MT], f32, tag="rot")
            nc.tensor.matmul(out=pr[:], lhsT=pswap[:], rhs=qkw[:],
                             start=True, stop=True)

            # sin/cos
            pg = ps_pg.tile([DH, MT], f32, tag="pg")
            nc.tensor.matmul(out=pg[:], lhsT=grep[:],
                             rhs=pos_fb[bi][:, tl0:tl0 + MT],
                             start=True, stop=True)
            tmag = sbp.tile([DH, MT], f32, tag="tmag")
            nc.scalar.activation(out=tmag[:], in_=pg[:], func=Act.Copy,
                                 bias=MAGIC)
            mrnd = sbp.tile([DH, MT], f32, tag="mrnd")
            nc.vector.tensor_scalar(out=mrnd[:], in0=tmag[:], scalar1=-MAGIC,
                                    scalar2=None, op0=Alu.add)
            frac = sbp.tile([DH, MT], f32, tag="frac")
            nc.vector.tensor_tensor(out=frac[:], in0=pg[:], in1=mrnd[:],
                                    op=Alu.subtract)
            sint = scp.tile([DH, MT], f32, tag="sin")
            nc.scalar.activation(out=sint[:], in_=frac[:], func=Act.Sin,
                                 scale=TWO_PI, bias=zerob[:])
            afrac = sbp.tile([DH, MT], f32, tag="afrac")
            nc.scalar.activation(out=afrac[:], in_=frac[:], func=Act.Abs,
                                 bias=zerob[:])
            cost = scp.tile([DH, MT], f32, tag="cos")
            nc.scalar.activation(out=cost[:], in_=afrac[:], func=Act.Sin,
                                 scale=-TWO_PI, bias=pihalf[:])

            # rope = qkw*cos + rot*sin  -> feature-major table slice
            t1 = sbp.tile([DH, MT], f32, tag="t1")
            nc.vector.tensor_tensor(out=t1[:], in0=qkw[:], in1=cost[:],
                                    op=Alu.mult)
            t2 = sbp.tile([DH, MT], f32, tag="t2")
            nc.vector.tensor_tensor(out=t2[:], in0=pr[:], in1=sint[:],
                                    op=Alu.mult)
            rp = sbp.tile([DH, MT], f32, tag="rope")
            nc.vector.tensor_tensor(out=rp[:], in0=t1[:], in1=t2[:],
                                    op=Alu.add)

            # [q|q] and [k|k] table slices via selector matmuls
            pf2 = ps_fm2.tile([DH, MT], f32, tag="fm2")
            nc.tensor.matmul(out=pf2[:], lhsT=sqq[:], rhs=rp[:],
                             start=True, stop=True)
            nc.scalar.copy(out=fqq[bi][:, tl0:tl0 + MT], in_=pf2[:])
            pf3 = ps_fm2.tile([DH, MT], f32, tag="fm3")
            nc.tensor.matmul(out=pf3[:], lhsT=skk[:], rhs=rp[:],
                             start=True, stop=True)
            nc.scalar.copy(out=fkk[bi][:, tl0:tl0 + MT], in_=pf3[:])

            # ---- per-batch tail: SBUF gathers + dots ----
            if m % (NM // BC) == (NM // BC) - 1:
                idxs = gth.tile([128, 2 * (S // 16)], i16, tag="idxs")
                nc.sync.dma_start(
                    out=idxs[:].rearrange("p (g s) -> p g s", g=2),
                    in_=idx16[bi].rearrange("g p s -> p g s"))
                msk1 = gth.tile([1, S], f32, tag="msk1")
                nc.sync.dma_start(out=msk1[:], in_=msk[bi:bi + 1, :])

                def gather(dst_ap, src_ap, idx_ap):
                    nc.gpsimd.ap_gather(
                        out_ap=dst_ap.rearrange("c (n d) -> c n d", d=1),
                        in_ap=src_ap.rearrange("c (n d) -> c n d", d=1),
                        idxs_ap=idx_ap,
                        channels=DH, num_elems=S, d=1, num_idxs=S)

                SW = S // 16
                ga = gth.tile([DH, S], f32, tag="ga")   # [q[i0] | q[i1]]
                gather(ga[:], fqq[bi][:], idxs[:, 0:SW])
                gb = gth.tile([DH, S], f32, tag="gb")   # [k[i2] | k[i3]]
                gather(gb[:], fkk[bi][:], idxs[:, SW:2 * SW])

                prod = gth.tile([DH, S], f32, tag="prod")
                nc.vector.tensor_tensor(out=prod[:], in0=ga[:],
                                        in1=gb[:], op=Alu.mult)

                lgsb = gth.tile([1, S], f32, tag="lgsb")
                for c in range(S // MT):
                    pl = ps_lg.tile([1, MT], f32, tag="lg")
                    nc.tensor.matmul(out=pl[:], lhsT=ones[:],
                                     rhs=prod[:, MT * c:MT * (c + 1)],
                                     start=True, stop=True)
                    # fused /sqrt(D): logits * 0.125
                    nc.scalar.activation(out=lgsb[:, MT * c:MT * (c + 1)],
                                         in_=pl[:], func=Act.Copy, scale=0.125)
                nc.scalar.activation(out=msk1[:], in_=msk1[:], func=Act.Copy,
                                     scale=1.25e11, bias=-1.25e11)
                nc.vector.tensor_tensor(out=lgsb[:], in0=lgsb[:], in1=msk1[:],
                                        op=Alu.add)
                nc.sync.dma_start(out=out[bi:bi + 1, :], in_=lgsb[:])

    nc.compile()
    return nc


def _get_nc():
    if "nc" not in _CACHE:
        _CACHE["nc"] = _build_nc()
    return _CACHE["nc"]


def _shard(inputs):
    lhs = np.ascontiguousarray(np.asarray(inputs["last_hidden_state"], np.float32))
    w = np.ascontiguousarray(np.asarray(inputs["W"], np.float32))
    b = np.ascontiguousarray(np.asarray(inputs["b"], np.float32))
    pos = np.ascontiguousarray(np.asarray(inputs["position_ids"], np.int32))
    rel = np.ascontiguousarray(np.asarray(inputs["relations_idx"], np.int32))
    msk = np.ascontiguousarray(np.asarray(inputs["labels_mask"], np.float32))
    in_maps = []
    for c in range(NCORES):
        sl = slice(BC * c, BC * (c + 1))
        relc = rel[sl]
        # wrapped index layout for ap_gather: [type, 128, S//16] int16;
        # partition 16c+j holds indices i = s*16+j (replica per Q7 core)
        wrp = np.tile(relc.transpose(0, 2, 1)            # [BC, 4, S]
                      .reshape(BC, 4, S // 16, 16)       # [BC, 4, s, j]
                      .transpose(0, 1, 3, 2),            # [BC, 4, j, s]
                      (1, 1, 4, 1)).astype(np.int16)     # [BC, 4, 64, S//16]
        idx16c = np.ascontiguousarray(np.stack(
            [np.concatenate([wrp[:, 0], wrp[:, 1]], axis=1),    # [i0 | i1]
             np.concatenate([wrp[:, 2], wrp[:, 3]], axis=1)],   # [i2 | i3]
            axis=1))                                     # [BC, 2, 128, S//16]
        in_maps.append({
            "x": lhs[sl].reshape(T, H).copy(),
            "w": w,
            "bvec": b,
            "pos": pos[sl].copy(),
            "idx16": idx16c,
            "msk": msk[sl].copy(),
        })
    return in_maps


def kernel(**inputs):
    from concourse import bass_utils
    nc = _get_nc()
    in_maps = _shard(inputs)
    res = bass_utils.run_bass_kernel_spmd(
        nc, in_maps, core_ids=list(range(NCORES)))
    _CACHE["last_results"] = res
    outs = [res.results[c]["out"].reshape(BC, S) for c in range(NCORES)]
    return np.concatenate(outs, axis=0).astype(np.float32)

